# revision 26
# baseline (speedup 1.0000x reference)
"""AgentSelfAttention1d Trainium2 kernel.

Problem (per batch b of 8, one NeuronCore each):
    xt = x[b].T                       # [L=4096, D=512]
    q/k/v = xt @ W{q,k,v}.T + b       # [L, D]
    a  = AdaptiveAvgPool(q) -> [P=128, D]
    c  = softmax(a @ k.T, -1) @ v     # [P, D]
    r  = softmax(q @ a.T, -1) @ c     # [L, D]
    out[b] = r.T                      # [D, L]

Algebraic restructuring used here (everything channel-first on chip):
    apT[d,p]  = (Wq @ pool(x) / 32) + bq          "agent" tokens, [D, P]
    S1[p,l]   = sum_e H[e,p] x[e,l],  H[e,p] = sum_d Wk[d,e] apT[d,p]
                (k projection eliminated; bk drops out of softmax-1)
    E1        = exp(S1 - 10)                      free-axis softmax numerator
    M1[p,e]   = sum_l E1[p,l] x[e,l]   via PE-transposed E1 and x tiles
    c[p,d]    = (M1 @ Wv.T) / rowsum1[p] + bv     (v projection eliminated)
    S2T[p,l]  = sum_e G[e,p] x[e,l] + hq[p],  G from Wq like H,
                hq[p] = bq . a[p]                 (q projection eliminated)
    E2        = exp(S2T - 40);  colsum2[l] via PE ones-matmul
    out[d,l]  = (sum_p c[p,d] E2n[p,l]),  E2n = E2 * (1/colsum2) broadcast

All matmuls run in float32r (full-speed fp32 mode, ~1e-4 relative rounding).
Softmaxes use constant shifts instead of max-subtraction (logit absmax is
~21 / ~42 for this model; exp stays far inside fp32 range either way).
x is transposed on-chip with PE transpose-mode (saves 8 MB of HBM traffic
vs shipping x.T from the host).
"""

import numpy as np

import concourse.bass as bass
import concourse.mybir as mybir
import concourse.tile as tile
from concourse import bacc
from concourse.bass_utils import run_bass_kernel_spmd

F32R = mybir.dt.float32r
F32 = mybir.dt.float32

B, D, L, P = 8, 512, 4096, 128
KT = D // 128      # 4 contraction tiles of 128
NCH = L // 512     # 8 l-chunks of 512
NLT = L // 128     # 32 l-tiles of 128
SHIFT1 = 10.0      # constant logit shift, stage 1 (|S1| ~ 21)
SHIFT2 = 40.0      # constant logit shift, stage 2 (|S2| ~ 42)

_CACHE = {}


def build():
    nc = bacc.Bacc(target_bir_lowering=False, trn_type="TRN2")
    X = nc.dram_tensor("x", [D, L], F32R, kind="ExternalInput")
    WQT = nc.dram_tensor("WqT", [D, D], F32R, kind="ExternalInput")   # [e, d]
    WQN = nc.dram_tensor("Wqn", [D, D], F32R, kind="ExternalInput")   # [d, e]
    WKN = nc.dram_tensor("Wkn", [D, D], F32R, kind="ExternalInput")   # [d, e]
    WVT = nc.dram_tensor("WvT", [D, D], F32R, kind="ExternalInput")   # [e, d]
    BQC = nc.dram_tensor("bqc", [D, 2], F32R, kind="ExternalInput")   # [bq, 0]
    BQF = nc.dram_tensor("bqf", [D], F32, kind="ExternalInput")
    IDN = nc.dram_tensor("ident", [128, 128], F32R, kind="ExternalInput")
    ONE = nc.dram_tensor("ones128", [128, 128], F32R, kind="ExternalInput")
    BVF = nc.dram_tensor("bvf", [D], F32, kind="ExternalInput")
    OUT = nc.dram_tensor("out", [D, L], F32, kind="ExternalOutput")

    from contextlib import ExitStack
    with nc.allow_low_precision("float32r matmul operands"), \
         tile.TileContext(nc) as tc, ExitStack() as stack:
        sb = stack.enter_context(tc.tile_pool(name="sb", bufs=1))
        xtp = stack.enter_context(tc.tile_pool(name="xtp", bufs=6))
        outp = stack.enter_context(tc.tile_pool(name="outp", bufs=8))
        # PSUM budget (8 banks): s:2 + tp:2 + small:1 + acc:1 + rt:2
        psS = stack.enter_context(tc.tile_pool(name="psS", bufs=1, space="PSUM"))
        psC = stack.enter_context(tc.tile_pool(name="psC", bufs=1, space="PSUM"))
        psT = stack.enter_context(tc.tile_pool(name="psT", bufs=3, space="PSUM"))
        psA = stack.enter_context(tc.tile_pool(name="psA", bufs=1, space="PSUM"))
        psR = stack.enter_context(tc.tile_pool(name="psR", bufs=2, space="PSUM"))

        # ---- ACT warmup: pull the activation-table load to t=0 ---------------
        warm = sb.tile([128, 1], F32)
        nc.vector.memset(warm, 0.0)
        nc.scalar.activation(out=warm, in_=warm,
                             func=mybir.ActivationFunctionType.Exp,
                             bias=warm, scale=1.0)

        # ---- x + pipelined pooling (highest DMA priority) --------------------
        x_sb = sb.tile([128, KT, L], F32R)
        xp = sb.tile([128, KT, P], F32R)
        xr = X.rearrange("(k p) l -> p k l", p=128)
        HL = L // 2
        HS = P // 2
        for k in range(KT):
            nsplit = 2 if k < KT - 1 else 4
            PL, PS = L // nsplit, P // nsplit
            for h in range(nsplit):
                nc.sync.dma_start(
                    out=x_sb[:, k, bass.ts(h, PL)], in_=xr[:, k, bass.ts(h, PL)])
                nc.vector.reduce_sum(
                    out=xp[:, k, bass.ts(h, PS)],
                    in_=x_sb[:, k, bass.ts(h, PL)].rearrange(
                        "p (s t) -> p s t", t=L // P),
                    axis=mybir.AxisListType.X)

        # ---- startup-chain weights: apT needs wqt, S1 needs H needs wkn ------
        wqt = sb.tile([128, KT, D], F32R)
        wqn = sb.tile([128, KT, D], F32R)
        wkn = sb.tile([128, KT, D], F32R)
        wvt = sb.tile([128, KT, D], F32R)
        nc.sync.dma_start(out=wqt, in_=WQT.rearrange("(k p) e -> p k e", p=128))
        bqf = sb.tile([128, KT], F32)
        nc.sync.dma_start(out=bqf, in_=BQF.rearrange("(k p) -> p k", p=128))
        bqc = sb.tile([128, KT, 2], F32R)
        nc.sync.dma_start(out=bqc, in_=BQC.rearrange("(k p) t -> p k t", p=128))
        nc.sync.dma_start(out=wkn, in_=WKN.rearrange("(k p) e -> p k e", p=128))
        nc.sync.dma_start(out=wqn, in_=WQN.rearrange("(k p) e -> p k e", p=128))

        # ---- late-needed weights / constants ---------------------------------
        ident = sb.tile([128, 128], F32R)
        nc.sync.dma_start(out=ident, in_=IDN[:, :])
        nc.sync.dma_start(out=wvt, in_=WVT.rearrange("(k p) e -> p k e", p=128))
        ones128 = sb.tile([128, 128], F32R)
        nc.sync.dma_start(out=ones128, in_=ONE[:, :])
        bvf = sb.tile([128, KT], F32)
        nc.sync.dma_start(out=bvf, in_=BVF.rearrange("(k p) -> p k", p=128))
        sh1 = sb.tile([128, 1], F32)
        nc.vector.memset(sh1, -SHIFT1)
        sh2 = sb.tile([128, 1], F32)
        nc.vector.memset(sh2, -SHIFT2)

        # ---- agent tokens apT = Wq @ pool(x)/32 + bq -------------------------
        apt = sb.tile([128, KT, P], F32R)
        for d in range(KT):
            ps = psR.tile([128, P], F32, tag="rt")
            for k in range(KT):
                nc.tensor.matmul(ps, wqt[:, k, bass.ts(d, 128)], xp[:, k, :],
                                 start=(k == 0), stop=(k == KT - 1))
            nc.scalar.activation(
                out=apt[:, d, :], in_=ps,
                func=mybir.ActivationFunctionType.Identity,
                bias=bqf[:, d:d + 1], scale=1.0 / (L // P))

        # ---- G/H (weights contracted against agents), hq ---------------------
        g_sb = sb.tile([128, KT, P], F32R)
        h_sb = sb.tile([128, KT, P], F32R)
        for e in range(KT):
            ps = psR.tile([128, P], F32, tag="rt")
            for k in range(KT):
                nc.tensor.matmul(ps, wkn[:, k, bass.ts(e, 128)], apt[:, k, :],
                                 start=(k == 0), stop=(k == KT - 1))
            if e % 2 == 0:
                nc.scalar.copy(h_sb[:, e, :], ps)
            else:
                nc.vector.tensor_copy(h_sb[:, e, :], ps)
        for e in range(KT):
            ps = psR.tile([128, P], F32, tag="rt")
            for k in range(KT):
                nc.tensor.matmul(ps, wqn[:, k, bass.ts(e, 128)], apt[:, k, :],
                                 start=(k == 0), stop=(k == KT - 1))
            nc.scalar.copy(g_sb[:, e, :], ps)
        hps = psR.tile([128, 2], F32, tag="rt")
        for k in range(KT):
            nc.tensor.matmul(hps, apt[:, k, :], bqc[:, k, :],
                             start=(k == 0), stop=(k == KT - 1))
        hq = sb.tile([128, 2], F32)
        nc.scalar.activation(out=hq, in_=hps,
                             func=mybir.ActivationFunctionType.Identity,
                             bias=sh2, scale=1.0)

        e1 = sb.tile([128, NCH, 512], F32R)
        rs1 = sb.tile([128, NCH], F32)

        # ---- E1/x transposes + M1 = E1 @ x.T ---------------------------------
        # per group of 4 l-tiles: one [128,512] psum collects 4 E1 transposes;
        # per l-tile: one [128,512] psum collects 4 x transposes (-> x.T tile).
        e1t = sb.tile([128, NLT, 128], F32R)
        e2 = sb.tile([128, NCH, 512], F32R)
        m1ps = psA.tile([128, D], F32, tag="acc")
        alt = 0
        for a in range(NLT // 4):
            # stage-1 chunk a: scores + exp (accumulating row sums)
            ps1 = psS.tile([128, 512], F32, tag="s")
            for k in range(KT):
                nc.tensor.matmul(ps1, h_sb[:, k, :], x_sb[:, k, bass.ts(a, 512)],
                                 start=(k == 0), stop=(k == KT - 1))
            nc.scalar.activation(out=e1[:, a, :], in_=ps1,
                                 func=mybir.ActivationFunctionType.Exp,
                                 bias=sh1, scale=1.0,
                                 accum_out=rs1[:, a:a + 1])
            # E1 transposes for this chunk + x transposes + M1 accumulation
            eps = psT.tile([128, 512], F32R, tag="tp")
            for u in range(4):
                nc.tensor.transpose(eps[:, bass.ts(u, 128)],
                                    e1[:, a, bass.ts(u, 128)], ident)
            nc.scalar.copy(e1t[:, 4 * a:4 * (a + 1), :], eps)
            for u in range(4):
                j = 4 * a + u
                xps = psT.tile([128, 512], F32R, tag="tp")
                for k in range(KT):
                    nc.tensor.transpose(xps[:, bass.ts(k, 128)],
                                        x_sb[:, k, bass.ts(j, 128)], ident)
                xt_t = xtp.tile([128, D], F32R, tag="xt")
                if alt % 5 < 2:
                    nc.vector.tensor_copy(xt_t, xps)
                else:
                    nc.scalar.copy(xt_t, xps)
                alt += 1
                nc.tensor.matmul(m1ps, e1t[:, j, :], xt_t,
                                 start=(j == 0), stop=(j == NLT - 1))
            # stage-2 chunk a: scores, exp, colsum+broadcast, normalise
            ps = psR.tile([128, 512], F32, tag="rt")
            for k in range(KT):
                nc.tensor.matmul(ps, g_sb[:, k, :], x_sb[:, k, bass.ts(a, 512)],
                                 start=(k == 0), stop=(k == KT - 1))
            nc.scalar.activation(out=e2[:, a, :], in_=ps,
                                 func=mybir.ActivationFunctionType.Exp,
                                 bias=hq[:, 0:1], scale=1.0)
            csps = psC.tile([128, 512], F32, tag="cs")
            nc.tensor.matmul(csps, ones128, e2[:, a, :], start=True, stop=True)
            rb = outp.tile([128, 512], F32, tag="rb")
            nc.vector.reciprocal(rb, csps)
            nc.vector.tensor_mul(e2[:, a, :], e2[:, a, :], rb)
        m1 = sb.tile([128, D], F32R)
        nc.scalar.copy(m1, m1ps)
        rsum1 = sb.tile([128, 1], F32)
        nc.vector.reduce_sum(out=rsum1, in_=rs1, axis=mybir.AxisListType.X)
        inv1 = sb.tile([128, 1], F32)
        nc.vector.reciprocal(inv1, rsum1)

        # ---- c = (M1 @ WvT)/rowsum1 + bv --------------------------------------
        m1t = sb.tile([128, KT, 128], F32R)
        mps = psT.tile([128, 512], F32R, tag="tp")
        for i in range(KT):
            nc.tensor.transpose(mps[:, bass.ts(i, 128)], m1[:, bass.ts(i, 128)],
                                ident)
        nc.scalar.copy(m1t, mps)
        cps = psA.tile([128, D], F32, tag="acc")
        for i in range(KT):
            nc.tensor.matmul(cps, m1t[:, i, :], wvt[:, i, :],
                             start=(i == 0), stop=(i == KT - 1))
        # bv is NOT added to c here: stage-2 softmax columns sum to 1, so
        # bv^T @ E2n == bv broadcast; it is applied as a per-partition bias
        # in the output copies instead.
        c_sb = sb.tile([128, D], F32R)
        nc.scalar.activation(out=c_sb, in_=cps,
                             func=mybir.ActivationFunctionType.Copy,
                             bias=0.0, scale=inv1)

        # ---- output stream: out[d,l] = c.T @ E2n (DMA-bound) -----------------
        for ch in range(NCH):
            for d in range(KT):
                rps = psR.tile([128, 512], F32, tag="rt")
                nc.tensor.matmul(rps, c_sb[:, bass.ts(d, 128)], e2[:, ch, :],
                                 start=True, stop=True)
                o_t = outp.tile([128, 512], F32, tag="o")
                if (ch * KT + d) % 2 == 0:
                    nc.scalar.activation(
                        out=o_t, in_=rps,
                        func=mybir.ActivationFunctionType.Identity,
                        bias=bvf[:, d:d + 1], scale=1.0)
                else:
                    nc.vector.tensor_scalar_add(o_t, rps, bvf[:, d:d + 1])
                nc.sync.dma_start(
                    out=OUT[bass.ts(d, 128), bass.ts(ch, 512)], in_=o_t)


    nc.compile()
    return nc


def _host_inputs(x, Wq, bq, Wk, bk, Wv, bv):
    del bk  # stage-1 softmax is invariant to the k-projection bias
    common = {
        "WqT": np.ascontiguousarray(Wq.T),
        "Wqn": np.ascontiguousarray(Wq),
        "Wkn": np.ascontiguousarray(Wk),
        "WvT": np.ascontiguousarray(Wv.T),
        "bqc": np.ascontiguousarray(
            np.stack([bq, np.zeros_like(bq)], axis=1)),
        "bqf": np.ascontiguousarray(bq),
        "ident": np.eye(128, dtype=np.float32),
        "ones128": np.ones((128, 128), dtype=np.float32),
        "bvf": np.ascontiguousarray(bv),
    }
    maps = []
    for b in range(B):
        m = dict(common)
        m["x"] = np.ascontiguousarray(x[b])
        maps.append(m)
    return maps


def kernel(x, Wq, bq, Wk, bk, Wv, bv):
    x = np.asarray(x, dtype=np.float32)
    if "nc" not in _CACHE:
        _CACHE["nc"] = build()
    nc = _CACHE["nc"]
    in_maps = _host_inputs(x, np.asarray(Wq), np.asarray(bq), np.asarray(Wk),
                           np.asarray(bk), np.asarray(Wv), np.asarray(bv))
    res = run_bass_kernel_spmd(nc, in_maps, core_ids=list(range(B)))
    out = np.empty((B, D, L), dtype=np.float32)
    for b in range(B):
        out[b] = res.results[b]["out"]
    return out


# revision 34
# speedup vs baseline: 1.0486x; 1.0486x over previous
"""AgentSelfAttention1d Trainium2 kernel.

Problem (per batch b of 8, one NeuronCore each):
    xt = x[b].T                       # [L=4096, D=512]
    q/k/v = xt @ W{q,k,v}.T + b       # [L, D]
    a  = AdaptiveAvgPool(q) -> [P=128, D]
    c  = softmax(a @ k.T, -1) @ v     # [P, D]
    r  = softmax(q @ a.T, -1) @ c     # [L, D]
    out[b] = r.T                      # [D, L]

Algebraic restructuring used here (everything channel-first on chip):
    apT[d,p]  = (Wq @ pool(x) / 32) + bq          "agent" tokens, [D, P]
    S1[p,l]   = sum_e H[e,p] x[e,l],  H[e,p] = sum_d Wk[d,e] apT[d,p]
                (k projection eliminated; bk drops out of softmax-1)
    E1        = exp(S1 - 10)                      free-axis softmax numerator
    M1[p,e]   = sum_l E1[p,l] x[e,l]   via PE-transposed E1 and x tiles
    c[p,d]    = (M1 @ Wv.T) / rowsum1[p] + bv     (v projection eliminated)
    S2T[p,l]  = sum_e G[e,p] x[e,l] + hq[p],  G from Wq like H,
                hq[p] = bq . a[p]                 (q projection eliminated)
    E2        = exp(S2T - 40);  colsum2[l] via PE ones-matmul
    out[d,l]  = (sum_p c[p,d] E2n[p,l]),  E2n = E2 * (1/colsum2) broadcast

All matmuls run in float32r (full-speed fp32 mode, ~1e-4 relative rounding).
Softmaxes use constant shifts instead of max-subtraction (logit absmax is
~21 / ~42 for this model; exp stays far inside fp32 range either way).
x is transposed on-chip with PE transpose-mode (saves 8 MB of HBM traffic
vs shipping x.T from the host).
"""

import numpy as np

import concourse.bass as bass
import concourse.mybir as mybir
import concourse.tile as tile
from concourse import bacc
from concourse.bass_utils import run_bass_kernel_spmd

F32R = mybir.dt.float32r
F32 = mybir.dt.float32

B, D, L, P = 8, 512, 4096, 128
KT = D // 128      # 4 contraction tiles of 128
NCH = L // 512     # 8 l-chunks of 512
NLT = L // 128     # 32 l-tiles of 128
SHIFT1 = 10.0      # constant logit shift, stage 1 (|S1| ~ 21)
SHIFT2 = 40.0      # constant logit shift, stage 2 (|S2| ~ 42)

_CACHE = {}


def build():
    nc = bacc.Bacc(target_bir_lowering=False, trn_type="TRN2")
    X = nc.dram_tensor("x", [D, L], F32R, kind="ExternalInput")
    WQT = nc.dram_tensor("WqT", [D, D], F32R, kind="ExternalInput")   # [e, d]
    WQN = nc.dram_tensor("Wqn", [D, D], F32R, kind="ExternalInput")   # [d, e]
    WKN = nc.dram_tensor("Wkn", [D, D], F32R, kind="ExternalInput")   # [d, e]
    WVT = nc.dram_tensor("WvT", [D, D], F32R, kind="ExternalInput")   # [e, d]
    BQC = nc.dram_tensor("bqc", [D, 2], F32R, kind="ExternalInput")   # [bq, 0]
    BQF = nc.dram_tensor("bqf", [D], F32, kind="ExternalInput")
    IDN = nc.dram_tensor("ident", [128, 128], F32R, kind="ExternalInput")
    ONE = nc.dram_tensor("ones128", [128, 128], F32R, kind="ExternalInput")
    BVF = nc.dram_tensor("bvf", [D], F32, kind="ExternalInput")
    OUT = nc.dram_tensor("out", [D, L], F32, kind="ExternalOutput")

    from contextlib import ExitStack
    with nc.allow_low_precision("float32r matmul operands"), \
         tile.TileContext(nc) as tc, ExitStack() as stack:
        sb = stack.enter_context(tc.tile_pool(name="sb", bufs=1))
        xtp = stack.enter_context(tc.tile_pool(name="xtp", bufs=32))
        e1p = stack.enter_context(tc.tile_pool(name="e1p", bufs=2))
        wnp = stack.enter_context(tc.tile_pool(name="wnp", bufs=1))
        e1tp = stack.enter_context(tc.tile_pool(name="e1tp", bufs=2))
        outp = stack.enter_context(tc.tile_pool(name="outp", bufs=4))
        rbp = stack.enter_context(tc.tile_pool(name="rbp", bufs=2))
        # PSUM budget (8 banks): s:2 + tp:2 + small:1 + acc:1 + rt:2
        psS = stack.enter_context(tc.tile_pool(name="psS", bufs=2, space="PSUM"))
        psC = stack.enter_context(tc.tile_pool(name="psC", bufs=1, space="PSUM"))
        psT = stack.enter_context(tc.tile_pool(name="psT", bufs=2, space="PSUM"))
        psA = stack.enter_context(tc.tile_pool(name="psA", bufs=1, space="PSUM"))
        psR = stack.enter_context(tc.tile_pool(name="psR", bufs=2, space="PSUM"))

        # ---- ACT warmup: pull the activation-table load to t=0 ---------------
        warm = sb.tile([128, 1], F32)
        nc.vector.memset(warm, 0.0)
        nc.scalar.activation(out=warm, in_=warm,
                             func=mybir.ActivationFunctionType.Exp,
                             bias=warm, scale=1.0)

        # ---- x chunk-major + pooling + eager x-transposes --------------------
        # Chunk-major arrival means every x.T tile becomes transposable the
        # moment its chunk lands, so the PE fills the input-DMA window.
        ident = sb.tile([128, 128], F32R)
        nc.sync.dma_start(out=ident, in_=IDN[:, :])
        x_sb = sb.tile([128, KT, L], F32R)
        xp = sb.tile([128, KT, P], F32R)
        xr = X.rearrange("(k p) l -> p k l", p=128)
        SEG = P // NCH
        xt_tiles = []
        alt = 0
        for ch in range(NCH):
            nc.sync.dma_start(
                out=x_sb[:, :, bass.ts(ch, 512)], in_=xr[:, :, bass.ts(ch, 512)])
            nc.vector.reduce_sum(
                out=xp[:, :, bass.ts(ch, SEG)],
                in_=x_sb[:, :, bass.ts(ch, 512)].rearrange(
                    "p k (s t) -> p k s t", t=L // P),
                axis=mybir.AxisListType.X)
            for u in range(4):
                j = 4 * ch + u
                xps = psT.tile([128, 512], F32R, tag="tp")
                for k in range(KT):
                    nc.tensor.transpose(xps[:, bass.ts(k, 128)],
                                        x_sb[:, k, bass.ts(j, 128)], ident)
                xt_t = xtp.tile([128, D], F32R, tag="xt")
                if alt % 4 == 3:
                    nc.vector.tensor_copy(xt_t, xps)
                else:
                    nc.scalar.copy(xt_t, xps)
                alt += 1
                xt_tiles.append(xt_t)

        # ---- startup-chain weights: apT needs wqt, S1 needs H needs wkn ------
        wqt = sb.tile([128, KT, D], F32R)
        wvt = sb.tile([128, KT, D], F32R)
        nc.sync.dma_start(out=wqt, in_=WQT.rearrange("(k p) e -> p k e", p=128))
        bqf = sb.tile([128, KT], F32)
        nc.sync.dma_start(out=bqf, in_=BQF.rearrange("(k p) -> p k", p=128))
        bqc = sb.tile([128, KT, 2], F32R)
        nc.sync.dma_start(out=bqc, in_=BQC.rearrange("(k p) t -> p k t", p=128))
        wkn = wnp.tile([128, KT, D], F32R, tag="wn")
        nc.sync.dma_start(out=wkn, in_=WKN.rearrange("(k p) e -> p k e", p=128))
        nc.sync.dma_start(out=wvt, in_=WVT.rearrange("(k p) e -> p k e", p=128))
        ones128 = sb.tile([128, 128], F32R)
        nc.sync.dma_start(out=ones128, in_=ONE[:, :])
        bvf = sb.tile([128, KT], F32)
        nc.sync.dma_start(out=bvf, in_=BVF.rearrange("(k p) -> p k", p=128))
        sh1 = sb.tile([128, 1], F32)
        nc.vector.memset(sh1, -SHIFT1)
        sh2 = sb.tile([128, 1], F32)
        nc.vector.memset(sh2, -SHIFT2)

        # ---- agent tokens apT = Wq @ pool(x)/32 + bq -------------------------
        apt = sb.tile([128, KT, P], F32R)
        for d in range(KT):
            ps = psR.tile([128, P], F32, tag="rt")
            for k in range(KT):
                nc.tensor.matmul(ps, wqt[:, k, bass.ts(d, 128)], xp[:, k, :],
                                 start=(k == 0), stop=(k == KT - 1))
            nc.scalar.activation(
                out=apt[:, d, :], in_=ps,
                func=mybir.ActivationFunctionType.Identity,
                bias=bqf[:, d:d + 1], scale=1.0 / (L // P))

        # ---- G/H (weights contracted against agents), hq ---------------------
        g_sb = sb.tile([128, KT, P], F32R)
        h_sb = sb.tile([128, KT, P], F32R)
        for e in range(KT):
            ps = psR.tile([128, P], F32, tag="rt")
            for k in range(KT):
                nc.tensor.matmul(ps, wkn[:, k, bass.ts(e, 128)], apt[:, k, :],
                                 start=(k == 0), stop=(k == KT - 1))
            if e % 2 == 0:
                nc.scalar.copy(h_sb[:, e, :], ps)
            else:
                nc.vector.tensor_copy(h_sb[:, e, :], ps)
        wqn = wnp.tile([128, KT, D], F32R, tag="wn")
        nc.sync.dma_start(out=wqn, in_=WQN.rearrange("(k p) e -> p k e", p=128))
        for e in range(KT):
            ps = psR.tile([128, P], F32, tag="rt")
            for k in range(KT):
                nc.tensor.matmul(ps, wqn[:, k, bass.ts(e, 128)], apt[:, k, :],
                                 start=(k == 0), stop=(k == KT - 1))
            nc.scalar.copy(g_sb[:, e, :], ps)
        hps = psR.tile([128, 2], F32, tag="rt")
        for k in range(KT):
            nc.tensor.matmul(hps, apt[:, k, :], bqc[:, k, :],
                             start=(k == 0), stop=(k == KT - 1))
        hq = sb.tile([128, 2], F32)
        nc.scalar.activation(out=hq, in_=hps,
                             func=mybir.ActivationFunctionType.Identity,
                             bias=sh2, scale=1.0)

        rs1 = sb.tile([128, NCH], F32)

        # ---- E1/x transposes + M1 = E1 @ x.T ---------------------------------
        # per group of 4 l-tiles: one [128,512] psum collects 4 E1 transposes;
        # per l-tile: one [128,512] psum collects 4 x transposes (-> x.T tile).
        e2 = sb.tile([128, NCH, 512], F32R)
        m1ps = psA.tile([128, D], F32, tag="acc")
        for a in range(NLT // 4):
            # stage-1 chunk a: scores + exp (accumulating row sums)
            ps1 = psS.tile([128, 512], F32, tag="s")
            for k in range(KT):
                nc.tensor.matmul(ps1, h_sb[:, k, :], x_sb[:, k, bass.ts(a, 512)],
                                 start=(k == 0), stop=(k == KT - 1))
            e1_t = e1p.tile([128, 512], F32R, tag="e1")
            nc.scalar.activation(out=e1_t, in_=ps1,
                                 func=mybir.ActivationFunctionType.Exp,
                                 bias=sh1, scale=1.0,
                                 accum_out=rs1[:, a:a + 1])
            # E1 transposes for this chunk + M1 accumulation (x.T prebuilt)
            eps = psT.tile([128, 512], F32R, tag="tp")
            for u in range(4):
                nc.tensor.transpose(eps[:, bass.ts(u, 128)],
                                    e1_t[:, bass.ts(u, 128)], ident)
            e1t_t = e1tp.tile([128, 4, 128], F32R, tag="e1t")
            nc.scalar.copy(e1t_t, eps)
            for u in range(4):
                j = 4 * a + u
                nc.tensor.matmul(m1ps, e1t_t[:, u, :], xt_tiles[j],
                                 start=(j == 0), stop=(j == NLT - 1))
            # stage-2 chunk a: scores, exp, colsum+broadcast, normalise
            ps = psR.tile([128, 512], F32, tag="rt")
            for k in range(KT):
                nc.tensor.matmul(ps, g_sb[:, k, :], x_sb[:, k, bass.ts(a, 512)],
                                 start=(k == 0), stop=(k == KT - 1))
            nc.scalar.activation(out=e2[:, a, :], in_=ps,
                                 func=mybir.ActivationFunctionType.Exp,
                                 bias=hq[:, 0:1], scale=1.0)
            csps = psC.tile([128, 512], F32, tag="cs")
            nc.tensor.matmul(csps, ones128, e2[:, a, :], start=True, stop=True)
            rb = rbp.tile([128, 512], F32, tag="rb")
            nc.vector.reciprocal(rb, csps)
            nc.vector.tensor_mul(e2[:, a, :], e2[:, a, :], rb)
        m1 = sb.tile([128, D], F32R)
        nc.scalar.copy(m1, m1ps)
        rsum1 = sb.tile([128, 1], F32)
        nc.vector.reduce_sum(out=rsum1, in_=rs1, axis=mybir.AxisListType.X)
        inv1 = sb.tile([128, 1], F32)
        nc.vector.reciprocal(inv1, rsum1)

        # ---- c = (M1 @ WvT)/rowsum1 + bv --------------------------------------
        m1t = sb.tile([128, KT, 128], F32R)
        mps = psT.tile([128, 512], F32R, tag="tp")
        for i in range(KT):
            nc.tensor.transpose(mps[:, bass.ts(i, 128)], m1[:, bass.ts(i, 128)],
                                ident)
        nc.scalar.copy(m1t, mps)
        cps = psA.tile([128, D], F32, tag="acc")
        for i in range(KT):
            nc.tensor.matmul(cps, m1t[:, i, :], wvt[:, i, :],
                             start=(i == 0), stop=(i == KT - 1))
        # bv is NOT added to c here: stage-2 softmax columns sum to 1, so
        # bv^T @ E2n == bv broadcast; it is applied as a per-partition bias
        # in the output copies instead.
        c_sb = sb.tile([128, D], F32R)
        nc.scalar.activation(out=c_sb, in_=cps,
                             func=mybir.ActivationFunctionType.Copy,
                             bias=0.0, scale=inv1)

        # ---- output stream: out[d,l] = c.T @ E2n (DMA-bound) -----------------
        for ch in range(NCH):
            for d in range(KT):
                rps = psR.tile([128, 512], F32, tag="rt")
                nc.tensor.matmul(rps, c_sb[:, bass.ts(d, 128)], e2[:, ch, :],
                                 start=True, stop=True)
                o_t = outp.tile([128, 512], F32, tag="o")
                if (ch * KT + d) % 2 == 0:
                    nc.scalar.activation(
                        out=o_t, in_=rps,
                        func=mybir.ActivationFunctionType.Identity,
                        bias=bvf[:, d:d + 1], scale=1.0)
                else:
                    nc.vector.tensor_scalar_add(o_t, rps, bvf[:, d:d + 1])
                nc.sync.dma_start(
                    out=OUT[bass.ts(d, 128), bass.ts(ch, 512)], in_=o_t)


    nc.compile()
    return nc


def _host_inputs(x, Wq, bq, Wk, bk, Wv, bv):
    del bk  # stage-1 softmax is invariant to the k-projection bias
    common = {
        "WqT": np.ascontiguousarray(Wq.T),
        "Wqn": np.ascontiguousarray(Wq),
        "Wkn": np.ascontiguousarray(Wk),
        "WvT": np.ascontiguousarray(Wv.T),
        "bqc": np.ascontiguousarray(
            np.stack([bq, np.zeros_like(bq)], axis=1)),
        "bqf": np.ascontiguousarray(bq),
        "ident": np.eye(128, dtype=np.float32),
        "ones128": np.ones((128, 128), dtype=np.float32),
        "bvf": np.ascontiguousarray(bv),
    }
    maps = []
    for b in range(B):
        m = dict(common)
        m["x"] = np.ascontiguousarray(x[b])
        maps.append(m)
    return maps


def kernel(x, Wq, bq, Wk, bk, Wv, bv):
    x = np.asarray(x, dtype=np.float32)
    if "nc" not in _CACHE:
        _CACHE["nc"] = build()
    nc = _CACHE["nc"]
    in_maps = _host_inputs(x, np.asarray(Wq), np.asarray(bq), np.asarray(Wk),
                           np.asarray(bk), np.asarray(Wv), np.asarray(bv))
    res = run_bass_kernel_spmd(nc, in_maps, core_ids=list(range(B)))
    out = np.empty((B, D, L), dtype=np.float32)
    for b in range(B):
        out[b] = res.results[b]["out"]
    return out


# revision 39
# speedup vs baseline: 1.0531x; 1.0043x over previous
"""AgentSelfAttention1d Trainium2 kernel.

Problem (per batch b of 8, one NeuronCore each):
    xt = x[b].T                       # [L=4096, D=512]
    q/k/v = xt @ W{q,k,v}.T + b       # [L, D]
    a  = AdaptiveAvgPool(q) -> [P=128, D]
    c  = softmax(a @ k.T, -1) @ v     # [P, D]
    r  = softmax(q @ a.T, -1) @ c     # [L, D]
    out[b] = r.T                      # [D, L]

Algebraic restructuring used here (everything channel-first on chip):
    apT[d,p]  = (Wq @ pool(x) / 32) + bq          "agent" tokens, [D, P]
    S1[p,l]   = sum_e H[e,p] x[e,l],  H[e,p] = sum_d Wk[d,e] apT[d,p]
                (k projection eliminated; bk drops out of softmax-1)
    E1        = exp(S1 - 10)                      free-axis softmax numerator
    M1[p,e]   = sum_l E1[p,l] x[e,l]   via PE-transposed E1 and x tiles
    c[p,d]    = (M1 @ Wv.T) / rowsum1[p] + bv     (v projection eliminated)
    S2T[p,l]  = sum_e G[e,p] x[e,l] + hq[p],  G from Wq like H,
                hq[p] = bq . a[p]                 (q projection eliminated)
    E2        = exp(S2T - 40);  colsum2[l] via PE ones-matmul
    out[d,l]  = (sum_p c[p,d] E2n[p,l]),  E2n = E2 * (1/colsum2) broadcast

All matmuls run in float32r (full-speed fp32 mode, ~1e-4 relative rounding).
Softmaxes use constant shifts instead of max-subtraction (logit absmax is
~21 / ~42 for this model; exp stays far inside fp32 range either way).
x is transposed on-chip with PE transpose-mode (saves 8 MB of HBM traffic
vs shipping x.T from the host).
"""

import numpy as np

import concourse.bass as bass
import concourse.mybir as mybir
import concourse.tile as tile
from concourse import bacc
from concourse.bass_utils import run_bass_kernel_spmd

F32R = mybir.dt.float32r
F32 = mybir.dt.float32

B, D, L, P = 8, 512, 4096, 128
KT = D // 128      # 4 contraction tiles of 128
NCH = L // 512     # 8 l-chunks of 512
NLT = L // 128     # 32 l-tiles of 128
SHIFT1 = 10.0      # constant logit shift, stage 1 (|S1| ~ 21)
SHIFT2 = 40.0      # constant logit shift, stage 2 (|S2| ~ 42)

_CACHE = {}


def build():
    nc = bacc.Bacc(target_bir_lowering=False, trn_type="TRN2")
    X = nc.dram_tensor("x", [D, L], F32R, kind="ExternalInput")
    WQT = nc.dram_tensor("WqT", [D, D], F32R, kind="ExternalInput")   # [e, d]
    WQN = nc.dram_tensor("Wqn", [D, D], F32R, kind="ExternalInput")   # [d, e]
    WKN = nc.dram_tensor("Wkn", [D, D], F32R, kind="ExternalInput")   # [d, e]
    WVT = nc.dram_tensor("WvT", [D, D], F32R, kind="ExternalInput")   # [e, d]
    BQC = nc.dram_tensor("bqc", [D, 2], F32R, kind="ExternalInput")   # [bq, 0]
    BQF = nc.dram_tensor("bqf", [D], F32, kind="ExternalInput")
    IDN = nc.dram_tensor("ident", [128, 128], F32R, kind="ExternalInput")
    ONE = nc.dram_tensor("ones128", [128, 128], F32R, kind="ExternalInput")
    BVF = nc.dram_tensor("bvf", [D], F32, kind="ExternalInput")
    OUT = nc.dram_tensor("out", [D, L], F32, kind="ExternalOutput")

    from contextlib import ExitStack
    with nc.allow_low_precision("float32r matmul operands"), \
         tile.TileContext(nc) as tc, ExitStack() as stack:
        sb = stack.enter_context(tc.tile_pool(name="sb", bufs=1))
        xtp = stack.enter_context(tc.tile_pool(name="xtp", bufs=32))
        e1p = stack.enter_context(tc.tile_pool(name="e1p", bufs=2))
        wnp = stack.enter_context(tc.tile_pool(name="wnp", bufs=1))
        e1tp = stack.enter_context(tc.tile_pool(name="e1tp", bufs=2))
        outp = stack.enter_context(tc.tile_pool(name="outp", bufs=4))
        rbp = stack.enter_context(tc.tile_pool(name="rbp", bufs=2))
        # PSUM budget (8 banks): s:2 + tp:2 + small:1 + acc:1 + rt:2
        psS = stack.enter_context(tc.tile_pool(name="psS", bufs=2, space="PSUM"))
        psC = stack.enter_context(tc.tile_pool(name="psC", bufs=1, space="PSUM"))
        psT = stack.enter_context(tc.tile_pool(name="psT", bufs=2, space="PSUM"))
        psA = stack.enter_context(tc.tile_pool(name="psA", bufs=1, space="PSUM"))
        psR = stack.enter_context(tc.tile_pool(name="psR", bufs=2, space="PSUM"))

        # ---- ACT warmup: pull the activation-table load to t=0 ---------------
        warm = sb.tile([128, 1], F32)
        nc.vector.memset(warm, 0.0)
        nc.scalar.activation(out=warm, in_=warm,
                             func=mybir.ActivationFunctionType.Exp,
                             bias=warm, scale=1.0)

        # ---- x chunk-major + pooling + eager x-transposes --------------------
        # Chunk-major arrival means every x.T tile becomes transposable the
        # moment its chunk lands, so the PE fills the input-DMA window.
        ident = sb.tile([128, 128], F32R)
        nc.sync.dma_start(out=ident, in_=IDN[:, :])
        x_sb = sb.tile([128, KT, L], F32R)
        xp = sb.tile([128, KT, P], F32R)
        xr = X.rearrange("(k p) l -> p k l", p=128)
        SEG = P // NCH
        xt_tiles = []
        alt = 0
        for ch in range(NCH):
            nc.sync.dma_start(
                out=x_sb[:, :, bass.ts(ch, 512)], in_=xr[:, :, bass.ts(ch, 512)])
            nc.vector.reduce_sum(
                out=xp[:, :, bass.ts(ch, SEG)],
                in_=x_sb[:, :, bass.ts(ch, 512)].rearrange(
                    "p k (s t) -> p k s t", t=L // P),
                axis=mybir.AxisListType.X)
            for u in range(4):
                j = 4 * ch + u
                xps = psT.tile([128, 512], F32R, tag="tp")
                for k in range(KT):
                    nc.tensor.transpose(xps[:, bass.ts(k, 128)],
                                        x_sb[:, k, bass.ts(j, 128)], ident)
                xt_t = xtp.tile([128, D], F32R, tag="xt")
                if alt % 4 == 3:
                    nc.vector.tensor_copy(xt_t, xps)
                else:
                    nc.scalar.copy(xt_t, xps)
                alt += 1
                xt_tiles.append(xt_t)

        # ---- startup-chain weights: apT needs wqt, S1 needs H needs wkn ------
        wqt = sb.tile([128, KT, D], F32R)
        wvt = sb.tile([128, KT, D], F32R)
        nc.sync.dma_start(out=wqt, in_=WQT.rearrange("(k p) e -> p k e", p=128))
        bqf = sb.tile([128, KT], F32)
        nc.sync.dma_start(out=bqf, in_=BQF.rearrange("(k p) -> p k", p=128))
        bqc = sb.tile([128, KT, 2], F32R)
        nc.sync.dma_start(out=bqc, in_=BQC.rearrange("(k p) t -> p k t", p=128))
        wkn = wnp.tile([128, KT, D], F32R, tag="wn")
        nc.sync.dma_start(out=wkn, in_=WKN.rearrange("(k p) e -> p k e", p=128))
        nc.sync.dma_start(out=wvt, in_=WVT.rearrange("(k p) e -> p k e", p=128))
        ones128 = sb.tile([128, 128], F32R)
        nc.sync.dma_start(out=ones128, in_=ONE[:, :])
        bvf = sb.tile([128, KT], F32)
        nc.sync.dma_start(out=bvf, in_=BVF.rearrange("(k p) -> p k", p=128))
        sh1 = sb.tile([128, 1], F32)
        nc.vector.memset(sh1, -SHIFT1)
        sh2 = sb.tile([128, 1], F32)
        nc.vector.memset(sh2, -SHIFT2)

        # ---- agent tokens apT = Wq @ pool(x)/32 + bq -------------------------
        apt = sb.tile([128, KT, P], F32R)
        for d in range(KT):
            ps = psR.tile([128, P], F32, tag="rt")
            for k in range(KT):
                nc.tensor.matmul(ps, wqt[:, k, bass.ts(d, 128)], xp[:, k, :],
                                 start=(k == 0), stop=(k == KT - 1))
            nc.scalar.activation(
                out=apt[:, d, :], in_=ps,
                func=mybir.ActivationFunctionType.Identity,
                bias=bqf[:, d:d + 1], scale=1.0 / (L // P))

        # ---- G/H (weights contracted against agents), hq ---------------------
        g_sb = sb.tile([128, KT, P], F32R)
        h_sb = sb.tile([128, KT, P], F32R)
        for e in range(KT):
            ps = psR.tile([128, P], F32, tag="rt")
            for k in range(KT):
                nc.tensor.matmul(ps, wkn[:, k, bass.ts(e, 128)], apt[:, k, :],
                                 start=(k == 0), stop=(k == KT - 1))
            if e % 2 == 0:
                nc.scalar.copy(h_sb[:, e, :], ps)
            else:
                nc.vector.tensor_copy(h_sb[:, e, :], ps)
        wqn = wnp.tile([128, KT, D], F32R, tag="wn")
        nc.sync.dma_start(out=wqn, in_=WQN.rearrange("(k p) e -> p k e", p=128))
        for e in range(KT):
            ps = psR.tile([128, P], F32, tag="rt")
            for k in range(KT):
                nc.tensor.matmul(ps, wqn[:, k, bass.ts(e, 128)], apt[:, k, :],
                                 start=(k == 0), stop=(k == KT - 1))
            nc.scalar.copy(g_sb[:, e, :], ps)
        hps = psR.tile([128, 2], F32, tag="rt")
        for k in range(KT):
            nc.tensor.matmul(hps, apt[:, k, :], bqc[:, k, :],
                             start=(k == 0), stop=(k == KT - 1))
        hq = sb.tile([128, 2], F32)
        nc.scalar.activation(out=hq, in_=hps,
                             func=mybir.ActivationFunctionType.Identity,
                             bias=sh2, scale=1.0)

        rs1 = sb.tile([128, NCH], F32)

        # ---- E1/x transposes + M1 = E1 @ x.T ---------------------------------
        # per group of 4 l-tiles: one [128,512] psum collects 4 E1 transposes;
        # per l-tile: one [128,512] psum collects 4 x transposes (-> x.T tile).
        e2 = sb.tile([128, NCH, 512], F32R)
        m1ps = psA.tile([128, D], F32, tag="acc")
        for a in range(NLT // 4):
            # stage-1 chunk a: scores + exp (accumulating row sums)
            ps1 = psS.tile([128, 512], F32, tag="s")
            for k in range(KT):
                nc.tensor.matmul(ps1, h_sb[:, k, :], x_sb[:, k, bass.ts(a, 512)],
                                 start=(k == 0), stop=(k == KT - 1))
            e1_t = e1p.tile([128, 512], F32R, tag="e1")
            nc.scalar.activation(out=e1_t, in_=ps1,
                                 func=mybir.ActivationFunctionType.Exp,
                                 bias=sh1, scale=1.0,
                                 accum_out=rs1[:, a:a + 1])
            # E1 transposes for this chunk + M1 accumulation (x.T prebuilt)
            eps = psT.tile([128, 512], F32R, tag="tp")
            for u in range(4):
                nc.tensor.transpose(eps[:, bass.ts(u, 128)],
                                    e1_t[:, bass.ts(u, 128)], ident)
            e1t_t = e1tp.tile([128, 4, 128], F32R, tag="e1t")
            nc.vector.tensor_copy(e1t_t, eps)
            for u in range(4):
                j = 4 * a + u
                nc.tensor.matmul(m1ps, e1t_t[:, u, :], xt_tiles[j],
                                 start=(j == 0), stop=(j == NLT - 1))
            # stage-2 chunk a first: its 5-stage chain overlaps the rest
            ps = psR.tile([128, 512], F32, tag="rt")
            for k in range(KT):
                nc.tensor.matmul(ps, g_sb[:, k, :], x_sb[:, k, bass.ts(a, 512)],
                                 start=(k == 0), stop=(k == KT - 1))
            nc.scalar.activation(out=e2[:, a, :], in_=ps,
                                 func=mybir.ActivationFunctionType.Exp,
                                 bias=hq[:, 0:1], scale=1.0)
            csps = psC.tile([128, 512], F32, tag="cs")
            nc.tensor.matmul(csps, ones128, e2[:, a, :], start=True, stop=True)
            rb = rbp.tile([128, 512], F32, tag="rb")
            nc.vector.reciprocal(rb, csps)
            nc.vector.tensor_mul(e2[:, a, :], e2[:, a, :], rb)
        m1 = sb.tile([128, D], F32R)
        nc.scalar.copy(m1, m1ps)
        rsum1 = sb.tile([128, 1], F32)
        nc.vector.reduce_sum(out=rsum1, in_=rs1, axis=mybir.AxisListType.X)
        inv1 = sb.tile([128, 1], F32)
        nc.vector.reciprocal(inv1, rsum1)

        # ---- c = (M1 @ WvT)/rowsum1 + bv --------------------------------------
        m1t = sb.tile([128, KT, 128], F32R)
        mps = psT.tile([128, 512], F32R, tag="tp")
        for i in range(KT):
            nc.tensor.transpose(mps[:, bass.ts(i, 128)], m1[:, bass.ts(i, 128)],
                                ident)
        nc.scalar.copy(m1t, mps)
        cps = psA.tile([128, D], F32, tag="acc")
        for i in range(KT):
            nc.tensor.matmul(cps, m1t[:, i, :], wvt[:, i, :],
                             start=(i == 0), stop=(i == KT - 1))
        # bv is NOT added to c here: stage-2 softmax columns sum to 1, so
        # bv^T @ E2n == bv broadcast; it is applied as a per-partition bias
        # in the output copies instead.
        c_sb = sb.tile([128, D], F32R)
        nc.scalar.activation(out=c_sb, in_=cps,
                             func=mybir.ActivationFunctionType.Copy,
                             bias=0.0, scale=inv1)

        # ---- output stream: out[d,l] = c.T @ E2n (DMA-bound) -----------------
        for ch in range(NCH):
            for d in range(KT):
                rps = psR.tile([128, 512], F32, tag="rt")
                nc.tensor.matmul(rps, c_sb[:, bass.ts(d, 128)], e2[:, ch, :],
                                 start=True, stop=True)
                o_t = outp.tile([128, 512], F32, tag="o")
                if (ch * KT + d) % 2 == 0:
                    nc.scalar.activation(
                        out=o_t, in_=rps,
                        func=mybir.ActivationFunctionType.Identity,
                        bias=bvf[:, d:d + 1], scale=1.0)
                else:
                    nc.vector.tensor_scalar_add(o_t, rps, bvf[:, d:d + 1])
                nc.sync.dma_start(
                    out=OUT[bass.ts(d, 128), bass.ts(ch, 512)], in_=o_t)


    nc.compile()
    return nc


def _host_inputs(x, Wq, bq, Wk, bk, Wv, bv):
    del bk  # stage-1 softmax is invariant to the k-projection bias
    common = {
        "WqT": np.ascontiguousarray(Wq.T),
        "Wqn": np.ascontiguousarray(Wq),
        "Wkn": np.ascontiguousarray(Wk),
        "WvT": np.ascontiguousarray(Wv.T),
        "bqc": np.ascontiguousarray(
            np.stack([bq, np.zeros_like(bq)], axis=1)),
        "bqf": np.ascontiguousarray(bq),
        "ident": np.eye(128, dtype=np.float32),
        "ones128": np.ones((128, 128), dtype=np.float32),
        "bvf": np.ascontiguousarray(bv),
    }
    maps = []
    for b in range(B):
        m = dict(common)
        m["x"] = np.ascontiguousarray(x[b])
        maps.append(m)
    return maps


def kernel(x, Wq, bq, Wk, bk, Wv, bv):
    x = np.asarray(x, dtype=np.float32)
    if "nc" not in _CACHE:
        _CACHE["nc"] = build()
    nc = _CACHE["nc"]
    in_maps = _host_inputs(x, np.asarray(Wq), np.asarray(bq), np.asarray(Wk),
                           np.asarray(bk), np.asarray(Wv), np.asarray(bv))
    res = run_bass_kernel_spmd(nc, in_maps, core_ids=list(range(B)))
    out = np.empty((B, D, L), dtype=np.float32)
    for b in range(B):
        out[b] = res.results[b]["out"]
    return out
